# revision 4
# baseline (speedup 1.0000x reference)
"""Local cross-attention (kNN) Trainium2 Bass kernel — host-pregather design.

Math identity used: gather commutes with the linear K/V projections, so the
host gathers RAW key_features rows per (query, neighbor) slot (pure data
movement — knn is an input known at call time) and the device projects the
gathered slots with weight-stationary matmuls. This removes all indirect DMA.

Slot layout is q-outer: slot = q*K + k, so per-query neighbor groups are
contiguous and the k-reductions are dense.

Per core (data-parallel over queries, 5000 q/core -> 40 tiles of 128):
  per tile (4096 slots), in 1024-slot chunks (16 queries each):
    psK/psV = Wk^T/Wv^T @ rawT chunk       (PE)
    prod    = psK * Q-broadcast            (DVE)
    scores  = headmask^T @ prod            (PE, per-head sums; 512-chunks)
    ee      = exp(scores)                  (ACT; max-subtract skipped,
                                            scores are bounded ~|2|)
    wb      = M8 @ ee (per-head broadcast) (PE + ACT copy to SBUF)
    p2      = psV * wb                     (DVE)
    att     = reduce_k p2  (contig)        (DVE, per-chunk 16-query slices)
    den     = reduce_k wb  (contig, = per-head-replicated softmax denom)
    attn    = att * recip(den)             (DVE)
    out     = Wo^T @ attn + bo'            (PE + ACT)
Bias handling (exact): bk drops (softmax-invariant per (q,h) shift);
bv folds into bo' = bo + bv @ Wo on host; bq added on-device via ACT.
"""

import numpy as np
import ml_dtypes

N1, N2, D, H, K = 40000, 60000, 128, 8, 32
HD = D // H
SCALE = HD ** -0.5
NCORES = 8
N1C = N1 // NCORES          # 5000 queries per core
QT = 128                    # queries per tile
N1P = 5120                  # padded queries per core -> 40 tiles
NT = N1P // QT
S = K * QT                  # 4096 slots per tile
CH = 512                    # slots per K/V PSUM chunk (16 queries)
NCH = S // CH               # 8 chunks per tile
QCH = CH // K               # 16 queries per chunk
SC = 512                    # slots per score/wb chunk
NSC = CH // SC              # 1 score-chunk per chunk

_PROG = None


def _build():
    import concourse.bass as bass
    import concourse.tile as tile
    from concourse import bacc, mybir
    from contextlib import ExitStack

    f32 = mybir.dt.float32
    bf16 = mybir.dt.bfloat16
    AX = mybir.AxisListType
    OP = mybir.AluOpType
    AF = mybir.ActivationFunctionType

    nc = bacc.Bacc("TRN2", target_bir_lowering=False, debug=False,
                   enable_asserts=True, num_devices=1)

    rawT = nc.dram_tensor("rawT", [D, NT * S], bf16, kind="ExternalInput").ap()
    qT = nc.dram_tensor("qT", [D, N1P], f32, kind="ExternalInput").ap()
    wq = nc.dram_tensor("wq", [D, D], f32, kind="ExternalInput").ap()
    wk = nc.dram_tensor("wk", [D, D], bf16, kind="ExternalInput").ap()
    wv = nc.dram_tensor("wv", [D, D], bf16, kind="ExternalInput").ap()
    wo = nc.dram_tensor("wo", [D, D], bf16, kind="ExternalInput").ap()
    hmask = nc.dram_tensor("hmask", [D, H], bf16, kind="ExternalInput").ap()
    m8 = nc.dram_tensor("m8", [H, D], bf16, kind="ExternalInput").ap()
    bqs = nc.dram_tensor("bqs", [D, 1], f32, kind="ExternalInput").ap()
    bo2 = nc.dram_tensor("bo2", [D, 1], f32, kind="ExternalInput").ap()
    outT = nc.dram_tensor("outT", [D, N1P], f32, kind="ExternalOutput").ap()

    with tile.TileContext(nc) as tc:
        with ExitStack() as cst:
            cp = cst.enter_context(tc.tile_pool(name="const", bufs=1))
            wq_s = cp.tile([D, D], f32, tag="wq")
            wk_s = cp.tile([D, D], bf16, tag="wk")
            wv_s = cp.tile([D, D], bf16, tag="wv")
            wo_s = cp.tile([D, D], bf16, tag="wo")
            hm_s = cp.tile([D, H], bf16, tag="hm")
            m8_s = cp.tile([H, D], bf16, tag="m8")
            bq_s = cp.tile([D, 1], f32, tag="bq")
            bo_s = cp.tile([D, 1], f32, tag="bo")
            for sb, dr in ((wq_s, wq), (wk_s, wk), (wv_s, wv), (wo_s, wo),
                           (hm_s, hmask), (m8_s, m8), (bq_s, bqs),
                           (bo_s, bo2)):
                nc.sync.dma_start(sb[:], dr)
            qT_s = cp.tile([D, N1P], f32, tag="qTs")
            nc.sync.dma_start(qT_s[:], qT)
            qs_all = cp.tile([D, N1P], bf16, tag="qsall")

            rp = cst.enter_context(tc.tile_pool(name="raw", bufs=2))
            sp = cst.enter_context(tc.tile_pool(name="small", bufs=2))
            pp = cst.enter_context(tc.tile_pool(name="prodp", bufs=3))
            wp = cst.enter_context(tc.tile_pool(name="wbp", bufs=2))
            pk = cst.enter_context(tc.tile_pool(name="ps_k", bufs=2,
                                                space="PSUM"))
            pv = cst.enter_context(tc.tile_pool(name="ps_v", bufs=2,
                                                space="PSUM"))
            pc = cst.enter_context(tc.tile_pool(name="ps_sc", bufs=1,
                                                space="PSUM"))
            pw = cst.enter_context(tc.tile_pool(name="ps_w", bufs=2,
                                                space="PSUM"))
            pj = cst.enter_context(tc.tile_pool(name="ps_j", bufs=1,
                                                space="PSUM"))

            for t in range(NT):
                psQ = pj.tile([D, QT], f32, tag="proj")
                nc.tensor.matmul(psQ[:], lhsT=wq_s[:],
                                 rhs=qT_s[:, bass.ts(t, QT)],
                                 start=True, stop=True)
                nc.scalar.activation(qs_all[:, bass.ts(t, QT)], psQ[:],
                                     AF.Identity, bias=bq_s[:, :],
                                     scale=SCALE)

            for t in range(NT):
                raw = rp.tile([D, S], bf16, tag="raw")
                nc.sync.dma_start(raw[:], rawT[:, bass.ts(t, S)])
                qs = qs_all[:, bass.ts(t, QT)]

                att = sp.tile([D, QT], f32, tag="att")
                den = sp.tile([D, QT], f32, tag="den")
                for c in range(NCH):
                    rawc = raw[:, bass.ts(c, CH)]
                    psK = pk.tile([D, CH], f32, tag="psK")
                    psV = pv.tile([D, CH], f32, tag="psV")
                    for e in range(NSC):
                        nc.tensor.matmul(psK[:, bass.ts(e, SC)], lhsT=wk_s[:],
                                         rhs=rawc[:, bass.ts(e, SC)],
                                         start=True, stop=True)
                        nc.tensor.matmul(psV[:, bass.ts(e, SC)], lhsT=wv_s[:],
                                         rhs=rawc[:, bass.ts(e, SC)],
                                         start=True, stop=True)
                    vt = wp.tile([D, CH], bf16, tag="vt")
                    nc.scalar.activation(vt[:], psV[:], AF.Copy)
                    prod = pp.tile([D, CH], bf16, tag="prod")
                    nc.vector.tensor_tensor(
                        out=prod[:].rearrange("p (q k) -> p q k", k=K),
                        in0=psK[:].rearrange("p (q k) -> p q k", k=K),
                        in1=qs[:, bass.ts(c, QCH)].unsqueeze(2)
                            .broadcast_to([D, QCH, K]),
                        op=OP.mult)
                    psS = pc.tile([H, SC], f32, tag="psS")
                    nc.tensor.matmul(psS[:], lhsT=hm_s[:], rhs=prod[:],
                                     start=True, stop=True)
                    ee = sp.tile([H, SC], bf16, tag="ee")
                    nc.scalar.activation(ee[:], psS[:], AF.Exp)
                    psW = pw.tile([D, SC], f32, tag="psW")
                    nc.tensor.matmul(psW[:], lhsT=m8_s[:], rhs=ee[:],
                                     start=True, stop=True)
                    p2 = pp.tile([D, CH], bf16, tag="p2")
                    nc.vector.tensor_tensor(
                        out=p2[:], in0=vt[:], in1=psW[:], op=OP.mult)
                    nc.vector.tensor_reduce(
                        out=att[:, bass.ts(c, QCH)],
                        in_=p2[:].rearrange("p (q k) -> p q k", k=K),
                        axis=AX.X, op=OP.add)
                    nc.vector.tensor_reduce(
                        out=den[:, bass.ts(c, QCH)],
                        in_=psW[:].rearrange("p (q k) -> p q k", k=K),
                        axis=AX.X, op=OP.add)

                rden = sp.tile([D, QT], f32, tag="rden")
                nc.vector.reciprocal(rden[:], den[:])
                attn = sp.tile([D, QT], bf16, tag="attn")
                nc.vector.tensor_tensor(out=attn[:], in0=att[:], in1=rden[:],
                                        op=OP.mult)
                psO = pj.tile([D, QT], f32, tag="proj")
                nc.tensor.matmul(psO[:], lhsT=wo_s[:], rhs=attn[:],
                                 start=True, stop=True)
                oT = sp.tile([D, QT], f32, tag="oT")
                nc.scalar.activation(oT[:], psO[:], AF.Identity,
                                     bias=bo_s[:, :])
                nc.sync.dma_start(outT[:, bass.ts(t, QT)], oT[:])

    nc.compile()
    return nc


def _get_prog():
    global _PROG
    if _PROG is None:
        _PROG = _build()
    return _PROG


def _host_inputs(query_features, key_features, knn_indices,
                 Wq, bq, Wk, bk, Wv, bv, Wo, bo):
    qf = np.asarray(query_features, np.float32)
    kf = np.asarray(key_features, np.float32)
    ki = np.asarray(knn_indices).astype(np.int64)

    kf_bf = kf.astype(ml_dtypes.bfloat16)
    wq_ = np.ascontiguousarray(np.asarray(Wq, np.float32))
    wk_ = np.ascontiguousarray(np.asarray(Wk, np.float32)).astype(
        ml_dtypes.bfloat16)
    wv_ = np.ascontiguousarray(np.asarray(Wv, np.float32)).astype(
        ml_dtypes.bfloat16)
    wo_ = np.ascontiguousarray(np.asarray(Wo, np.float32)).astype(
        ml_dtypes.bfloat16)
    hmask = np.zeros((D, H), ml_dtypes.bfloat16)
    for h in range(H):
        hmask[h * HD:(h + 1) * HD, h] = 1
    m8 = np.ascontiguousarray(hmask.T)
    bqs = (np.asarray(bq, np.float32) * SCALE).reshape(D, 1)
    bo2 = (np.asarray(bo, np.float32)
           + np.asarray(bv, np.float32) @ np.asarray(Wo, np.float32)
           ).reshape(D, 1)

    in_maps = []
    for c in range(NCORES):
        g = np.zeros((N1P, K, D), ml_dtypes.bfloat16)
        g[:N1C] = kf_bf[ki[c * N1C:(c + 1) * N1C]]
        # rawT[d, (t, q, k)] = g[t*128+q, k, d]
        rawTc = np.ascontiguousarray(
            g.reshape(NT, QT, K, D).transpose(3, 0, 1, 2)
        ).reshape(D, NT * S)
        qTc = np.zeros((D, N1P), np.float32)
        qTc[:, :N1C] = qf[c * N1C:(c + 1) * N1C].T
        in_maps.append({
            "rawT": rawTc, "qT": qTc,
            "wq": wq_, "wk": wk_, "wv": wv_, "wo": wo_,
            "hmask": hmask, "m8": m8, "bqs": bqs, "bo2": bo2,
        })
    return in_maps


def kernel(query_features, key_features, knn_indices,
           Wq, bq, Wk, bk, Wv, bv, Wo, bo):
    from concourse import bass_utils

    nc = _get_prog()
    in_maps = _host_inputs(query_features, key_features, knn_indices,
                           Wq, bq, Wk, bk, Wv, bv, Wo, bo)
    res = bass_utils.run_bass_kernel_spmd(
        nc, in_maps, core_ids=list(range(NCORES)))

    out = np.empty((N1, D), np.float32)
    for c in range(NCORES):
        out[c * N1C:(c + 1) * N1C] = res.results[c]["outT"][:, :N1C].T
    return out
